# revision 13
# baseline (speedup 1.0000x reference)
"""Chamfer distance kernel for Trainium2 (8 NeuronCores, data-parallel over batch).

Full inputs x, y: [8, 4096, 3] fp32. Output: [8] fp32.

v3 strategy (per core = one batch):
  dist(i,j)*512 computed on the PE as a K=38 fp8e4m3 DoubleRow matmul
  (0.5 cycles/output column -- 2x bf16 throughput):
    - coords scaled by 32; 4-level fp8 splits; cross rows for level pairs
      (i,j) with i+j<=3  -> 30 rows
    - norms scaled 512/128, 4-level fp8 splits, paired with const-128 rows
      -> 8 rows.  Numpy-validated rel err ~1e-4 (gate 2e-2).
  Two passes (x-on-partitions then y-on-partitions); each pass streams 32
  chunks of [128 pts, 4096 cols] through a 4-slot PSUM quad ring
  ([128,1024] fp32).  Min-reduction is split across all three consumer
  engines; C-chunks (ScalarE copies + DVE reduce) are quad-interleaved
  with a partner G-chunk (GpSimd pair-folds + DVE reduce) so ScalarE and
  GpSimd stream concurrently instead of alternating:
    - G-chunks (23/32): gpsimd tensor_tensor(min) folds quad pairs
      PSUM->SBUF; one DVE tensor_tensor_reduce finishes 2048 cols and
      drops the chunk min into an acc column via accum_out.
    - C-chunks (9/32): ScalarE copies the 4 quads PSUM->SBUF; one DVE
      tensor_tensor_reduce over [2048]+[2048] makes the acc column.
  Final: per-pass acc [128,32] summed on DVE -> [128,2] -> DRAM; host sums
  partitions and divides by 512*N.
"""

import os
import sys

import numpy as np

for _p in ("/opt/trn_rl_repo", "/root/.axon_site/_ro/trn_rl_repo"):
    if os.path.isdir(_p) and _p not in sys.path:
        sys.path.insert(0, _p)

B = 8
N = 4096
D = 3
P = 128
IPP = N // P      # 32 points per partition
NCH = N // P      # 32 chunks of 128 points
NLV = 4           # fp8 split levels
MAXSUM = 3        # keep cross pairs with i+j <= MAXSUM
SC = 32.0         # coord scale; u*v = -1024*x*y = 512*(-2xy)
CONST = 128.0     # norm pairing constant (exact in e4m3)
DSCALE = 512.0    # distance scale (= SC*SC/2); e4m3 max is 240
BIG = 3.0e38
QCOLS = 1024      # psum quad columns
N_C_CHUNKS = 9    # C-chunks per 32-chunk pass (rest are G-chunks)

_CACHE = {}

# cross pairs (i = lhs level, contiguous rhs levels 0..nr-1 per group)
PAIRS = []
for i in range(NLV):
    nr = sum(1 for j in range(NLV) if i + j <= MAXSUM)
    if nr:
        PAIRS.append((i, nr))
NCROSS = sum(nr for _, nr in PAIRS) * D   # 30
KROWS = NCROSS + 2 * NLV                  # 38
KPART = KROWS // 2                        # 19 rows per DoubleRow k-tile


def _build_nc():
    from contextlib import ExitStack

    from concourse import bacc, mybir, masks
    from concourse.tile import TileContext

    f32 = mybir.dt.float32
    bf16 = mybir.dt.bfloat16
    fp8 = mybir.dt.float8e4
    MIN = mybir.AluOpType.min
    ADD = mybir.AluOpType.add
    AX = mybir.AxisListType.X
    DR = mybir.MatmulPerfMode.DoubleRow

    nc = bacc.Bacc()
    x_d = nc.declare_dram_parameter("x", [N, D], f32, isOutput=False)
    y_d = nc.declare_dram_parameter("y", [N, D], f32, isOutput=False)
    res_d = nc.declare_dram_parameter("res", [P, 2], f32, isOutput=True)

    with ExitStack() as ctx:
        tc = ctx.enter_context(TileContext(nc))
        singles = ctx.enter_context(tc.tile_pool(name="singles", bufs=1))
        gjp = ctx.enter_context(tc.tile_pool(name="gj", bufs=3))
        csp = ctx.enter_context(tc.tile_pool(name="cs", bufs=3))
        jkp = ctx.enter_context(tc.tile_pool(name="jk", bufs=2))
        psum = ctx.enter_context(tc.tile_pool(name="psum", bufs=4, space="PSUM"))

        # ---- PE warmup: ramp the tensor engine while DMAs/splits run.
        wsrc = singles.tile([32, 512], bf16, tag="wsrc")
        nc.gpsimd.memset(wsrc[:], 1.0)
        for wi in range(8):
            wq = psum.tile([P, QCOLS], f32, tag="q")
            nc.tensor.matmul(
                wq[:, 0:512], wsrc[:, 0:128], wsrc[:], start=True, stop=True
            )

        ident8 = singles.tile([P, P], fp8, tag="ident8")
        masks.make_identity(nc, ident8[:])

        # ---- input DMAs (points on partitions: point j = p*IPP + i)
        raw = {}
        for side, dram in (("y", y_d), ("x", x_d)):
            r = singles.tile([P, IPP, D], f32, tag=f"raw_{side}")
            nc.sync.dma_start(
                out=r[:], in_=dram[:, :].rearrange("(p i) c -> p i c", p=P)
            )
            raw[side] = r

        # ---- per-side prep: fp8 level splits + W staging [P, IPP, 38]
        # y gates pass A: its chain uses ACT casts + DVE subs; x uses Pool.
        W = {}
        for side in ("y", "x"):
            r = raw[side]
            sub_eng = nc.vector if side == "y" else nc.gpsimd
            asm_eng = nc.vector if side == "y" else nc.gpsimd
            sgn = 1.0 if side == "x" else -1.0

            u = singles.tile([P, IPP, D], f32, tag=f"u_{side}")
            sub_eng.tensor_scalar_mul(u[:], r[:], sgn * SC)
            sq = singles.tile([P, IPP, D], f32, tag=f"sq_{side}")
            nc.gpsimd.tensor_mul(sq[:], r[:], r[:])
            nrm = singles.tile([P, IPP], f32, tag=f"nrm_{side}")
            nc.vector.tensor_reduce(nrm[:], sq[:], axis=AX, op=ADD)
            nsc = singles.tile([P, IPP], f32, tag=f"nsc_{side}")
            sub_eng.tensor_scalar_mul(nsc[:], nrm[:], DSCALE / CONST)

            lv = singles.tile([P, IPP, D, NLV], fp8, tag=f"lv_{side}")
            nl = singles.tile([P, IPP, NLV], fp8, tag=f"nl_{side}")
            cur = u
            curn = nsc
            for l in range(NLV):
                nc.scalar.copy(lv[:, :, :, l], cur[:])
                nc.scalar.copy(nl[:, :, l], curn[:])
                if l < NLV - 1:
                    nxt = singles.tile([P, IPP, D], f32, tag=f"r{l}_{side}")
                    sub_eng.tensor_sub(nxt[:], cur[:], lv[:, :, :, l])
                    cur = nxt
                    nxtn = singles.tile([P, IPP], f32, tag=f"rn{l}_{side}")
                    sub_eng.tensor_sub(nxtn[:], curn[:], nl[:, :, l])
                    curn = nxtn

            # W assembly on ACT (y) / Pool (x): keep DVE light for the
            # main loop.
            wasm = nc.scalar if side == "y" else nc.gpsimd

            def wcopy(dst, src):
                if wasm is nc.scalar:
                    wasm.copy(dst, src)
                else:
                    wasm.tensor_copy(dst, src)

            w = singles.tile([P, IPP, KROWS], fp8, tag=f"w_{side}")
            rr = 0
            for c in range(D):
                for i, nr in PAIRS:
                    if side == "x":
                        wcopy(
                            w[:, :, rr : rr + nr],
                            lv[:, :, c, i : i + 1].broadcast_to([P, IPP, nr]),
                        )
                    else:
                        wcopy(w[:, :, rr : rr + nr], lv[:, :, c, 0:nr])
                    rr += nr
            if side == "x":
                wcopy(w[:, :, NCROSS : NCROSS + NLV], nl[:])
                nc.gpsimd.memset(w[:, :, NCROSS + NLV : KROWS], CONST)
            else:
                nc.gpsimd.memset(w[:, :, NCROSS : NCROSS + NLV], CONST)
                wcopy(w[:, :, NCROSS + NLV : KROWS], nl[:])
            W[side] = w

        # ---- KM emission: PE-transpose W blocks into PSUM (fp8), copy to
        # SBUF KM [KPART, 2, N].  One psum quad slot stages half a side.
        KM = {}
        cp_engines = [nc.scalar, nc.gpsimd, nc.scalar, nc.vector]
        cp_i = [0]

        def emit_km(side):
            w = W[side]
            km = singles.tile([KPART, 2, N], fp8, tag=f"km_{side}")
            KM[side] = km
            for half in range(2):
                slot = psum.tile([P, QCOLS], f32, tag="q")
                tp = slot[:, :].bitcast(fp8).rearrange(
                    "p (t j) -> p t j", t=2
                )  # [128, 2, 2048] fp8 view
                for bb in range(16):
                    b = half * 16 + bb
                    for t in range(2):
                        nc.tensor.transpose(
                            tp[0:KPART, t, bb * P : (bb + 1) * P],
                            w[:, b, t * KPART : (t + 1) * KPART],
                            ident8[:],
                        )
                for cpb in range(2):
                    eng = cp_engines[cp_i[0] % len(cp_engines)]
                    cp_i[0] += 1
                    dst = km[
                        :, :,
                        half * 2048 + cpb * 1024 : half * 2048 + (cpb + 1) * 1024,
                    ]
                    src = tp[0:KPART, :, cpb * 1024 : (cpb + 1) * 1024]
                    if eng is nc.scalar:
                        eng.copy(dst, src)
                    else:
                        eng.tensor_copy(dst, src)

        emit_km("y")
        emit_km("x")

        # ---- main: two passes; C-chunks quad-interleaved with partner Gs
        rs_all = singles.tile([P, 2], f32, tag="rs_all")
        c_flags = [((i * N_C_CHUNKS) % NCH) < N_C_CHUNKS for i in range(NCH)]
        units = []  # ("G", c) or ("CG", c_c, c_g)
        ci = 0
        while ci < NCH:
            if c_flags[ci] and ci + 1 < NCH and not c_flags[ci + 1]:
                units.append(("CG", ci, ci + 1))
                ci += 2
            elif c_flags[ci]:
                units.append(("C", ci))
                ci += 1
            else:
                units.append(("G", ci))
                ci += 1

        for li, (lhs_side, rhs_side) in enumerate((("x", "y"), ("y", "x"))):
            lhs_km, rhs_km = KM[lhs_side], KM[rhs_side]
            acc = singles.tile([P, NCH], f32, tag=f"acc_{li}")

            def mm(q, lhsT, qi, mj):
                j0 = qi * QCOLS + mj * 512
                nc.tensor.matmul(
                    q[:, mj * 512 : (mj + 1) * 512],
                    lhsT,
                    rhs_km[:, :, j0 : j0 + 512],
                    start=True,
                    stop=True,
                    perf_mode=DR,
                )

            def consume_g(quads, c):
                gj = gjp.tile([P, 2 * QCOLS], f32, tag="gj")
                nc.gpsimd.tensor_tensor(
                    out=gj[:, 0:QCOLS], in0=quads[0][:], in1=quads[1][:], op=MIN
                )
                nc.gpsimd.tensor_tensor(
                    out=gj[:, QCOLS:], in0=quads[2][:], in1=quads[3][:], op=MIN
                )
                junk = jkp.tile([P, QCOLS], f32, tag="junk_g")
                nc.vector.tensor_tensor_reduce(
                    out=junk[:],
                    in0=gj[:, 0:QCOLS],
                    in1=gj[:, QCOLS:],
                    scale=1.0,
                    scalar=BIG,
                    op0=MIN,
                    op1=MIN,
                    accum_out=acc[:, c : c + 1],
                )

            def consume_c_final(cs, c):
                # bf16 fold tree on DVE (tensor_tensor runs in fast mode on
                # 2-byte SBUF data), then a small ttr for the accum column.
                h = jkp.tile([P, 2 * QCOLS], bf16, tag="h_c")
                nc.vector.tensor_tensor(
                    out=h[:], in0=cs[:, 0 : 2 * QCOLS], in1=cs[:, 2 * QCOLS :],
                    op=MIN,
                )
                h2 = jkp.tile([P, QCOLS], bf16, tag="h2_c")
                nc.vector.tensor_tensor(
                    out=h2[:], in0=h[:, 0:QCOLS], in1=h[:, QCOLS:], op=MIN
                )
                junk = jkp.tile([P, QCOLS // 2], f32, tag="junk_c")
                nc.vector.tensor_tensor_reduce(
                    out=junk[:],
                    in0=h2[:, 0 : QCOLS // 2],
                    in1=h2[:, QCOLS // 2 :],
                    scale=1.0,
                    scalar=BIG,
                    op0=MIN,
                    op1=MIN,
                    accum_out=acc[:, c : c + 1],
                )

            for unit in units:
                if unit[0] == "CG":
                    _, cc, cg = unit
                    lhsT_c = lhs_km[:, :, cc * P : (cc + 1) * P]
                    lhsT_g = lhs_km[:, :, cg * P : (cg + 1) * P]
                    cs = csp.tile([P, 4 * QCOLS], bf16, tag="cs")
                    gq = []
                    for qi in range(4):
                        qc = psum.tile([P, QCOLS], f32, tag="q")
                        for mj in range(2):
                            mm(qc, lhsT_c, qi, mj)
                        qg = psum.tile([P, QCOLS], f32, tag="q")
                        for mj in range(2):
                            mm(qg, lhsT_g, qi, mj)
                        gq.append(qg)
                        nc.scalar.copy(
                            cs[:, qi * QCOLS : (qi + 1) * QCOLS], qc[:]
                        )
                    consume_g(gq, cg)
                    consume_c_final(cs, cc)
                else:
                    kind, c = unit[0], unit[1]
                    lhsT = lhs_km[:, :, c * P : (c + 1) * P]
                    quads = []
                    for qi in range(4):
                        q = psum.tile([P, QCOLS], f32, tag="q")
                        quads.append(q)
                        for mj in range(2):
                            mm(q, lhsT, qi, mj)
                    if kind == "C":
                        cs = csp.tile([P, 4 * QCOLS], bf16, tag="cs")
                        for qi in range(4):
                            nc.scalar.copy(
                                cs[:, qi * QCOLS : (qi + 1) * QCOLS],
                                quads[qi][:],
                            )
                        consume_c_final(cs, c)
                    else:
                        consume_g(quads, c)
            nc.vector.tensor_reduce(
                rs_all[:, li : li + 1], acc[:], axis=AX, op=ADD
            )
        nc.sync.dma_start(out=res_d[:, :], in_=rs_all[:])

    if not nc.is_finalized():
        nc.finalize()
    return nc


def _get_nc():
    if "nc" not in _CACHE:
        _CACHE["nc"] = _build_nc()
    return _CACHE["nc"]


def _postprocess(results):
    out = np.empty(B, np.float32)
    for b in range(B):
        r = np.asarray(results[b]["res"], dtype=np.float64)  # [128, 2]
        out[b] = (r[:, 0].sum() + r[:, 1].sum()) / (N * DSCALE)
    return out


def kernel(x, y):
    from concourse.bass_utils import run_bass_kernel_spmd

    x = np.ascontiguousarray(np.asarray(x, dtype=np.float32))
    y = np.ascontiguousarray(np.asarray(y, dtype=np.float32))
    assert x.shape == (B, N, D) and y.shape == (B, N, D)
    nc = _get_nc()
    in_maps = [{"x": x[b], "y": y[b]} for b in range(B)]
    res = run_bass_kernel_spmd(nc, in_maps, core_ids=list(range(B)))
    return _postprocess(res.results)


def timed_run(x, y, **kwargs):
    """Run with NTFF tracing; returns (output, exec_time_ns)."""
    from concourse.bass_utils import run_bass_kernel_spmd

    x = np.ascontiguousarray(np.asarray(x, dtype=np.float32))
    y = np.ascontiguousarray(np.asarray(y, dtype=np.float32))
    nc = _get_nc()
    in_maps = [{"x": x[b], "y": y[b]} for b in range(B)]
    res = run_bass_kernel_spmd(
        nc, in_maps, core_ids=list(range(B)), trace=True, **kwargs
    )
    return _postprocess(res.results), res.exec_time_ns


# revision 17
# speedup vs baseline: 1.0127x; 1.0127x over previous
"""Chamfer distance kernel for Trainium2 (8 NeuronCores, data-parallel over batch).

Full inputs x, y: [8, 4096, 3] fp32. Output: [8] fp32.

v3 strategy (per core = one batch):
  dist(i,j)*512 computed on the PE as a K=38 fp8e4m3 DoubleRow matmul
  (0.5 cycles/output column -- 2x bf16 throughput):
    - coords scaled by 32; 4-level fp8 splits; cross rows for level pairs
      (i,j) with i+j<=3  -> 30 rows
    - norms scaled 512/128, 4-level fp8 splits, paired with const-128 rows
      -> 8 rows.  Numpy-validated rel err ~1e-4 (gate 2e-2).
  Two passes (x-on-partitions then y-on-partitions); each pass streams 32
  chunks of [128 pts, 4096 cols] through a 4-slot PSUM quad ring
  ([128,1024] fp32).  Min-reduction is split across all three consumer
  engines; C-chunks (ScalarE copies + DVE reduce) are quad-interleaved
  with a partner G-chunk (GpSimd pair-folds + DVE reduce) so ScalarE and
  GpSimd stream concurrently instead of alternating:
    - G-chunks (23/32): gpsimd tensor_tensor(min) folds quad pairs
      PSUM->SBUF; one DVE tensor_tensor_reduce finishes 2048 cols and
      drops the chunk min into an acc column via accum_out.
    - C-chunks (9/32): ScalarE copies the 4 quads PSUM->SBUF; one DVE
      tensor_tensor_reduce over [2048]+[2048] makes the acc column.
  Final: per-pass acc [128,32] summed on DVE -> [128,2] -> DRAM; host sums
  partitions and divides by 512*N.
"""

import os
import sys

import numpy as np

for _p in ("/opt/trn_rl_repo", "/root/.axon_site/_ro/trn_rl_repo"):
    if os.path.isdir(_p) and _p not in sys.path:
        sys.path.insert(0, _p)

B = 8
N = 4096
D = 3
P = 128
IPP = N // P      # 32 points per partition
NCH = N // P      # 32 chunks of 128 points
NLV = 4           # fp8 split levels
MAXSUM = 3        # keep cross pairs with i+j <= MAXSUM
SC = 32.0         # coord scale; u*v = -1024*x*y = 512*(-2xy)
CONST = 128.0     # norm pairing constant (exact in e4m3)
DSCALE = 512.0    # distance scale (= SC*SC/2); e4m3 max is 240
BIG = 3.0e38
QCOLS = 1024      # psum quad columns
N_C_CHUNKS = 9    # C-chunks per 32-chunk pass (rest are G-chunks)

_CACHE = {}

# cross pairs (i = lhs level, contiguous rhs levels 0..nr-1 per group)
PAIRS = []
for i in range(NLV):
    nr = sum(1 for j in range(NLV) if i + j <= MAXSUM)
    if nr:
        PAIRS.append((i, nr))
NCROSS = sum(nr for _, nr in PAIRS) * D   # 30
KROWS = NCROSS + 2 * NLV                  # 38
KPART = KROWS // 2                        # 19 rows per DoubleRow k-tile


def _build_nc():
    from contextlib import ExitStack

    from concourse import bacc, mybir, masks
    from concourse.tile import TileContext

    f32 = mybir.dt.float32
    bf16 = mybir.dt.bfloat16
    fp8 = mybir.dt.float8e4
    MIN = mybir.AluOpType.min
    ADD = mybir.AluOpType.add
    AX = mybir.AxisListType.X
    DR = mybir.MatmulPerfMode.DoubleRow

    nc = bacc.Bacc()
    x_d = nc.declare_dram_parameter("x", [N, D], f32, isOutput=False)
    y_d = nc.declare_dram_parameter("y", [N, D], f32, isOutput=False)
    res_d = nc.declare_dram_parameter("res", [P, 2], f32, isOutput=True)

    with ExitStack() as ctx:
        tc = ctx.enter_context(TileContext(nc))
        singles = ctx.enter_context(tc.tile_pool(name="singles", bufs=1))
        gjp = ctx.enter_context(tc.tile_pool(name="gj", bufs=3))
        csp = ctx.enter_context(tc.tile_pool(name="cs", bufs=3))
        jkp = ctx.enter_context(tc.tile_pool(name="jk", bufs=2))
        psum = ctx.enter_context(tc.tile_pool(name="psum", bufs=4, space="PSUM"))

        # ---- PE warmup: ramp the tensor engine while DMAs/splits run.
        wsrc = singles.tile([32, 512], bf16, tag="wsrc")
        nc.gpsimd.memset(wsrc[:], 1.0)
        for wi in range(8):
            wq = psum.tile([P, QCOLS], f32, tag="q")
            nc.tensor.matmul(
                wq[:, 0:512], wsrc[:, 0:128], wsrc[:], start=True, stop=True
            )

        ident8 = singles.tile([P, P], fp8, tag="ident8")
        masks.make_identity(nc, ident8[:])

        # ---- input DMAs (points on partitions: point j = p*IPP + i)
        raw = {}
        for side, dram in (("y", y_d), ("x", x_d)):
            r = singles.tile([P, IPP, D], f32, tag=f"raw_{side}")
            nc.sync.dma_start(
                out=r[:], in_=dram[:, :].rearrange("(p i) c -> p i c", p=P)
            )
            raw[side] = r

        # ---- per-side prep: fp8 level splits + W staging [P, IPP, 38]
        # y gates pass A: its chain uses ACT casts + DVE subs; x uses Pool.
        W = {}
        for side in ("y", "x"):
            r = raw[side]
            sub_eng = nc.vector if side == "y" else nc.gpsimd
            asm_eng = nc.vector if side == "y" else nc.gpsimd
            sgn = 1.0 if side == "x" else -1.0

            u = singles.tile([P, IPP, D], f32, tag=f"u_{side}")
            sub_eng.tensor_scalar_mul(u[:], r[:], sgn * SC)
            sq = singles.tile([P, IPP, D], f32, tag=f"sq_{side}")
            nc.gpsimd.tensor_mul(sq[:], r[:], r[:])
            nrm = singles.tile([P, IPP], f32, tag=f"nrm_{side}")
            nc.vector.tensor_reduce(nrm[:], sq[:], axis=AX, op=ADD)
            nsc = singles.tile([P, IPP], f32, tag=f"nsc_{side}")
            sub_eng.tensor_scalar_mul(nsc[:], nrm[:], DSCALE / CONST)

            lv = singles.tile([P, IPP, D, NLV], fp8, tag=f"lv_{side}")
            nl = singles.tile([P, IPP, NLV], fp8, tag=f"nl_{side}")
            cur = u
            curn = nsc
            for l in range(NLV):
                nc.scalar.copy(lv[:, :, :, l], cur[:])
                nc.scalar.copy(nl[:, :, l], curn[:])
                if l < NLV - 1:
                    nxt = singles.tile([P, IPP, D], f32, tag=f"r{l}_{side}")
                    sub_eng.tensor_sub(nxt[:], cur[:], lv[:, :, :, l])
                    cur = nxt
                    nxtn = singles.tile([P, IPP], f32, tag=f"rn{l}_{side}")
                    sub_eng.tensor_sub(nxtn[:], curn[:], nl[:, :, l])
                    curn = nxtn

            # W assembly on ACT (y) / Pool (x): keep DVE light for the
            # main loop.
            wasm = nc.scalar if side == "y" else nc.gpsimd

            def wcopy(dst, src):
                if wasm is nc.scalar:
                    wasm.copy(dst, src)
                else:
                    wasm.tensor_copy(dst, src)

            w = singles.tile([P, IPP, KROWS], fp8, tag=f"w_{side}")
            rr = 0
            for c in range(D):
                for i, nr in PAIRS:
                    if side == "x":
                        wcopy(
                            w[:, :, rr : rr + nr],
                            lv[:, :, c, i : i + 1].broadcast_to([P, IPP, nr]),
                        )
                    else:
                        wcopy(w[:, :, rr : rr + nr], lv[:, :, c, 0:nr])
                    rr += nr
            if side == "x":
                wcopy(w[:, :, NCROSS : NCROSS + NLV], nl[:])
                nc.gpsimd.memset(w[:, :, NCROSS + NLV : KROWS], CONST)
            else:
                nc.gpsimd.memset(w[:, :, NCROSS : NCROSS + NLV], CONST)
                wcopy(w[:, :, NCROSS + NLV : KROWS], nl[:])
            W[side] = w

        # ---- KM emission: PE-transpose W blocks into PSUM (fp8), copy to
        # SBUF KM [KPART, 2, N].  One psum quad slot stages half a side.
        KM = {}

        def emit_km(side):
            w = W[side]
            km = singles.tile([KPART, 2, N], fp8, tag=f"km_{side}")
            KM[side] = km
            for half in range(2):
                slot = psum.tile([P, QCOLS], f32, tag="q")
                tp = slot[:, :].bitcast(fp8).rearrange(
                    "p (t j) -> p t j", t=2
                )  # [128, 2, 2048] fp8 view
                for bb in range(16):
                    b = half * 16 + bb
                    for t in range(2):
                        nc.tensor.transpose(
                            tp[0:KPART, t, bb * P : (bb + 1) * P],
                            w[:, b, t * KPART : (t + 1) * KPART],
                            ident8[:],
                        )
                # uint16-bitcast copy: 2-byte packed view of the fp8 data
                # lets DVE run its 2x mode (and integers skip finite checks).
                u16 = mybir.dt.uint16
                for cpb in range(2):
                    dst = km[
                        :, :,
                        half * 2048 + cpb * 1024 : half * 2048 + (cpb + 1) * 1024,
                    ].bitcast(u16)
                    src = tp[
                        0:KPART, :, cpb * 1024 : (cpb + 1) * 1024
                    ].bitcast(u16)
                    nc.vector.tensor_copy(dst, src)

        emit_km("y")
        emit_km("x")

        # ---- main: two passes; C-chunks quad-interleaved with partner Gs,
        # plus a couple of DVE-direct D-chunks to offload GpSimd.
        rs_all = singles.tile([P, 2], f32, tag="rs_all")
        kind = {}
        for i in range(NCH):
            kind[i] = "C" if i % 4 == 0 else "G"
        kind[14] = "D"
        kind[30] = "D"
        units = []  # ("G", c) / ("CG", c_c, c_g) / ("D", c)
        ci = 0
        while ci < NCH:
            if kind[ci] == "C" and ci + 1 < NCH and kind[ci + 1] == "G":
                units.append(("CG", ci, ci + 1))
                ci += 2
            else:
                units.append((kind[ci], ci))
                ci += 1

        H = QCOLS // 2
        for li, (lhs_side, rhs_side) in enumerate((("x", "y"), ("y", "x"))):
            lhs_km, rhs_km = KM[lhs_side], KM[rhs_side]
            acc = singles.tile([P, NCH], f32, tag=f"acc_{li}")
            acc2 = singles.tile([P, NCH], f32, tag=f"acc2_{li}")
            nc.vector.memset(acc2[:], BIG)

            def mm(q, lhsT, qi, mj):
                j0 = qi * QCOLS + mj * 512
                nc.tensor.matmul(
                    q[:, mj * 512 : (mj + 1) * 512],
                    lhsT,
                    rhs_km[:, :, j0 : j0 + 512],
                    start=True,
                    stop=True,
                    perf_mode=DR,
                )

            def fold512(a, b, c, tag):
                # DVE bf16 fold tree finishing in the acc column: two
                # 512-wide tensor_tensor mins + one 512-pair ttr.
                u = jkp.tile([P, 2 * H], bf16, tag=f"u_{tag}")
                nc.vector.tensor_tensor(out=u[:, 0:H], in0=a[0], in1=a[1], op=MIN)
                nc.vector.tensor_tensor(out=u[:, H:], in0=b[0], in1=b[1], op=MIN)
                junk = jkp.tile([P, H], f32, tag=f"junk_{tag}")
                nc.vector.tensor_tensor_reduce(
                    out=junk[:],
                    in0=u[:, 0:H],
                    in1=u[:, H:],
                    scale=1.0,
                    scalar=BIG,
                    op0=MIN,
                    op1=MIN,
                    accum_out=acc[:, c : c + 1],
                )

            def consume_g(quads, c):
                gj = gjp.tile([P, 2 * QCOLS], bf16, tag="gj")
                nc.gpsimd.tensor_tensor(
                    out=gj[:, 0:QCOLS], in0=quads[0][:], in1=quads[1][:], op=MIN
                )
                nc.gpsimd.tensor_tensor(
                    out=gj[:, QCOLS:], in0=quads[2][:], in1=quads[3][:], op=MIN
                )
                fold512(
                    (gj[:, 0:H], gj[:, H:QCOLS]),
                    (gj[:, QCOLS : QCOLS + H], gj[:, QCOLS + H :]),
                    c,
                    "g",
                )

            def consume_c_final(cs, c):
                # level-1: 4096 -> 2048 with four 512-wide bf16 tts
                t = jkp.tile([P, 4 * H], bf16, tag="t_c")
                for k in range(4):
                    nc.vector.tensor_tensor(
                        out=t[:, k * H : (k + 1) * H],
                        in0=cs[:, k * QCOLS : k * QCOLS + H],
                        in1=cs[:, k * QCOLS + H : (k + 1) * QCOLS],
                        op=MIN,
                    )
                fold512(
                    (t[:, 0:H], t[:, H : 2 * H]),
                    (t[:, 2 * H : 3 * H], t[:, 3 * H :]),
                    c,
                    "c",
                )

            def consume_d(quads, c):
                # DVE-direct: two PSUM-pair ttrs into acc/acc2 columns
                for pi, dst in ((0, acc), (1, acc2)):
                    junk = jkp.tile([P, QCOLS], f32, tag=f"junk_d{pi}")
                    nc.vector.tensor_tensor_reduce(
                        out=junk[:],
                        in0=quads[2 * pi][:],
                        in1=quads[2 * pi + 1][:],
                        scale=1.0,
                        scalar=BIG,
                        op0=MIN,
                        op1=MIN,
                        accum_out=dst[:, c : c + 1],
                    )

            for unit in units:
                if unit[0] == "CG":
                    _, cc, cg = unit
                    lhsT_c = lhs_km[:, :, cc * P : (cc + 1) * P]
                    lhsT_g = lhs_km[:, :, cg * P : (cg + 1) * P]
                    cs = csp.tile([P, 4 * QCOLS], bf16, tag="cs")
                    gq = []
                    for qi in range(4):
                        qc = psum.tile([P, QCOLS], f32, tag="q")
                        for mj in range(2):
                            mm(qc, lhsT_c, qi, mj)
                        qg = psum.tile([P, QCOLS], f32, tag="q")
                        for mj in range(2):
                            mm(qg, lhsT_g, qi, mj)
                        gq.append(qg)
                        nc.scalar.copy(
                            cs[:, qi * QCOLS : (qi + 1) * QCOLS], qc[:]
                        )
                    consume_g(gq, cg)
                    consume_c_final(cs, cc)
                else:
                    knd, c = unit[0], unit[1]
                    lhsT = lhs_km[:, :, c * P : (c + 1) * P]
                    quads = []
                    for qi in range(4):
                        q = psum.tile([P, QCOLS], f32, tag="q")
                        quads.append(q)
                        for mj in range(2):
                            mm(q, lhsT, qi, mj)
                    if knd == "C":
                        cs = csp.tile([P, 4 * QCOLS], bf16, tag="cs")
                        for qi in range(4):
                            nc.scalar.copy(
                                cs[:, qi * QCOLS : (qi + 1) * QCOLS],
                                quads[qi][:],
                            )
                        consume_c_final(cs, c)
                    elif knd == "D":
                        consume_d(quads, c)
                    else:
                        consume_g(quads, c)
            rm = singles.tile([P, NCH], f32, tag=f"rm_{li}")
            nc.vector.tensor_tensor(out=rm[:], in0=acc[:], in1=acc2[:], op=MIN)
            nc.vector.tensor_reduce(
                rs_all[:, li : li + 1], rm[:], axis=AX, op=ADD
            )
        nc.sync.dma_start(out=res_d[:, :], in_=rs_all[:])

    if not nc.is_finalized():
        nc.finalize()
    return nc


def _get_nc():
    if "nc" not in _CACHE:
        _CACHE["nc"] = _build_nc()
    return _CACHE["nc"]


def _postprocess(results):
    out = np.empty(B, np.float32)
    for b in range(B):
        r = np.asarray(results[b]["res"], dtype=np.float64)  # [128, 2]
        out[b] = (r[:, 0].sum() + r[:, 1].sum()) / (N * DSCALE)
    return out


def kernel(x, y):
    from concourse.bass_utils import run_bass_kernel_spmd

    x = np.ascontiguousarray(np.asarray(x, dtype=np.float32))
    y = np.ascontiguousarray(np.asarray(y, dtype=np.float32))
    assert x.shape == (B, N, D) and y.shape == (B, N, D)
    nc = _get_nc()
    in_maps = [{"x": x[b], "y": y[b]} for b in range(B)]
    res = run_bass_kernel_spmd(nc, in_maps, core_ids=list(range(B)))
    return _postprocess(res.results)


def timed_run(x, y, **kwargs):
    """Run with NTFF tracing; returns (output, exec_time_ns)."""
    from concourse.bass_utils import run_bass_kernel_spmd

    x = np.ascontiguousarray(np.asarray(x, dtype=np.float32))
    y = np.ascontiguousarray(np.asarray(y, dtype=np.float32))
    nc = _get_nc()
    in_maps = [{"x": x[b], "y": y[b]} for b in range(B)]
    res = run_bass_kernel_spmd(
        nc, in_maps, core_ids=list(range(B)), trace=True, **kwargs
    )
    return _postprocess(res.results), res.exec_time_ns
